# revision 15
# baseline (speedup 1.0000x reference)
"""Trainium2 Bass kernel for masked attention-pooling (DmasifAttentionModule).

Reference computation (per sample b):
    proj   = x @ W.T + b                  # [N, D]
    scores = proj @ v                     # [N]
    scores = where(mask, scores, -1e9)
    w      = softmax(scores)              # [N]
    out    = w @ x                        # [D]

Optimizations (exact up to fp reassociation unless noted):
  1. scores = x @ (W.T @ v) + (b . v); softmax is shift-invariant, so the
     (b . v) constant drops out and the 34-GFLOP projection collapses to a
     matvec against u = v @ W (host-computed, 512 floats).
  2. Masked rows get softmax weight exactly 0, so only the ~50% valid rows
     participate. The host compacts each sample to its valid rows (padded
     to a common column count with zero rows) and streams only those.
  3. x and u ship as fp16: halves HBM traffic (the binding resource), runs
     the pooling matmul at full PE rate (fp32 = 4 passes) and keeps DVE
     elementwise ops in 2x_1p mode. Score accumulation stays fp32.
  4. The compacted shard is host-swizzled to partition-major
     [128, SPB, ncols, D] so the whole 4.45 MiB arrives as ONE dma_start
     with 17.4 KiB contiguous per partition (meas. 295 GB/s vs 245 for
     per-tile strided transfers). Double-buffered across For_i iterations.
  5. Scores: a DVE free-dim reduce only has a 1x uop (694 ns/[128,512]
     column) while plain tensor_tensor runs 2x (438 ns), so columns are
     split: ~half fused on DVE (scalar_tensor_tensor w/ accum), the rest
     as DVE 2x products + ScalarE Copy-with-accum reduce (872 ns, ScalarE
     is otherwise idle). Masking is a single posthoc [128,ncols]
     tensor_add of -3e8 per masked column (no fp16-range contortions).
  6. exp: one batched ScalarE activation per sample (bias = -C shift),
     fp16 out. No accum: Z is recovered on host from the e tensor itself
     (8.7 KiB DMA per core), so numerator and denominator use bit-identical
     weights.
  7. Pooling: TensorE matvec accumulation into PSUM [1,512] per sample
     (lhsT = e column [128,1] fp16, rhs = x column [128,512] fp16,
     216 ns each); ScalarE copies PSUM out, host divides by Z.

Per-core budgets at ncols=17 (8 cores, 2 samples each, data-parallel):
DMA ~15.1 us, DVE ~18.7 us, ACT ~19 us, PE ~8 us -> ~20 us/iter steady.
"""

import os
import sys

import numpy as np

for _p in ("/opt/trn_rl_repo", "/root/.axon_site/_ro/trn_rl_repo"):
    if os.path.isdir(_p) and _p not in sys.path:
        sys.path.append(_p)

import concourse.bacc as bacc
import concourse.tile as tile
from concourse import mybir
from concourse.bass_utils import run_bass_kernel_spmd

B, N, D = 16, 4096, 512
N_CORES = 8
SPB = B // N_CORES          # samples per core
C_SHIFT = 24.0              # constant exp-range shift (softmax-invariant)
MASKED_INIT = -3.0e8        # masked scores -> exp underflows to exactly 0
ACT_COLS = 8                # score columns per sample reduced on ScalarE

_F32 = mybir.dt.float32
_F16 = mybir.dt.float16
_CACHE = {}


def _build_program(ncols, loop_n=None, act_cols=None, mask_in_stt=None):
    """Program for samples compacted to `ncols` columns of 128 rows each.

    loop_n wraps the computation in a HW For_i loop (timing only).
    mask_in_stt is accepted for test.py compatibility and ignored."""
    if act_cols is None:
        act_cols = ACT_COLS
    na = min(act_cols, max(0, ncols - 1))   # ScalarE-reduced cols per sample
    # Contiguous runs of <=4 so the DVE product op covers a whole run
    # (one [128, 4*512] 2x tensor_tensor = 1224 ns vs 4x438 split).
    quads = [(c0, min(4, na - c0)) for c0 in range(0, na, 4)]

    nc = bacc.Bacc("TRN2", target_bir_lowering=False, debug=False)
    x = nc.dram_tensor("x", [128, SPB, ncols, D], _F16,
                       kind="ExternalInput").ap()
    u = nc.dram_tensor("u", [128, 4 * D], _F16, kind="ExternalInput").ap()
    out = nc.dram_tensor("out", [SPB, D], _F32, kind="ExternalOutput").ap()
    eout = nc.dram_tensor("eout", [128, SPB, ncols], _F16,
                          kind="ExternalOutput").ap()

    with tile.TileContext(nc) as tc:
        with (
            tc.tile_pool(name="xp", bufs=2) as xp,
            tc.tile_pool(name="singles", bufs=1) as sg,
            tc.tile_pool(name="prod", bufs=4) as prp,
            tc.tile_pool(name="scratch", bufs=2) as scr,
            tc.tile_pool(name="smalls", bufs=2) as sm,
            tc.tile_pool(name="ps", bufs=1, space="PSUM") as psp,
        ):
            ones_sb = sg.tile([128, 1], _F32)
            nc.vector.memset(ones_sb[:], 1.0)
            shift_sb = sg.tile([128, 1], _F32)
            nc.vector.memset(shift_sb[:], -C_SHIFT)
            warm = sg.tile([128, 1], _F32)
            # Pull the exp table-set load (~2.7us) to t=0, under the DMAs.
            nc.scalar.activation(warm[:], ones_sb[:],
                                 mybir.ActivationFunctionType.Exp)

            u_sb = sg.tile([128, 4, D], _F16)   # u replicated 4x along free
            nc.sync.dma_start(out=u_sb[:], in_=u.rearrange("p (r d) -> p r d",
                                                           r=4))

            # PSUM pooling accumulators for both unrolled halves, so each
            # half's finalize copies can be deferred into the other half
            # (by which time the pooling matmuls are long done -> no stall).
            ps = [{s: psp.tile([1, D], _F32, name=f"ps_{h}_{s}")
                   for s in range(SPB)} for h in range(2)]

            ctx = (nc, xp, prp, scr, sm, x, out, eout, u_sb,
                   shift_sb, ncols, quads, na, ps)

            if loop_n is not None:
                # For_i is a HW loop over a STATIC body: tile-pool rotation
                # only happens across emit calls, so double buffering needs
                # the body unrolled x2 (iteration i+1's DMA lands in the
                # other buffer and overlaps iteration i's compute).
                assert loop_n % 2 == 0, loop_n
                with tc.For_i(0, loop_n // 2, 1) as _i:
                    _emit_iteration(*ctx, half=0, fin_half=1)
                    _emit_iteration(*ctx, half=1, fin_half=0)
            else:
                _emit_iteration(*ctx, half=0, fin_half=0)

    nc.compile()
    return nc


def _emit_iteration(nc, xp, prp, scr, sm, x, out, eout, u_sb,
                    shift_sb, ncols, quads, na, ps, half, fin_half):
    # One DMA for the whole shard; double-buffered across the two unrolled
    # halves so the transfer of iteration i+1 overlaps compute of i.
    xt = xp.tile([128, SPB, ncols, D], _F16, name="xt")
    nc.sync.dma_start(out=xt[:], in_=x[:])
    # Per-emit score/e tiles (rotate with the unrolled halves) so the two
    # in-flight iterations never alias.
    s_sb = xp.tile([128, SPB, ncols], _F32, name="s_sb")
    e_sb = xp.tile([128, SPB, ncols], _F16, name="e_sb")
    pool_ps = ps[half]

    for s in range(SPB):
        # ScalarE-routed columns first: one DVE 2x product op per <=4-col
        # run, ScalarE Copy-accum reduces stream behind it per column.
        for c0, cw in quads:
            prod = prp.tile([128, cw, D], _F16, name=f"prod{c0}")
            nc.vector.tensor_tensor(
                out=prod[:], in0=xt[:, s, c0:c0 + cw, :],
                in1=u_sb[:, 0:cw, :], op=mybir.AluOpType.mult)
            for j in range(cw):
                dump32 = scr.tile([128, D], _F32, name="dump32")
                nc.scalar.activation(
                    dump32[:], prod[:, j, :],
                    mybir.ActivationFunctionType.Copy,
                    accum_out=s_sb[:, s, c0 + j:c0 + j + 1])
        # Remaining columns fused on DVE (1x scalar_tensor_tensor w/ accum).
        for c in range(na, ncols):
            dump = scr.tile([128, D], _F16, name="dump")
            nc.vector.scalar_tensor_tensor(
                out=dump[:], in0=xt[:, s, c, :],
                scalar=0.0, in1=u_sb[:, 0, :],
                op0=mybir.AluOpType.add, op1=mybir.AluOpType.mult,
                accum_out=s_sb[:, s, c:c + 1])
    # Masking needs no ops: the host writes padding rows as
    # x_pad = -kappa*u/||u||^2, so their score is exactly -kappa (exp -> 0)
    # and their pooling contribution is e*x = 0*x_pad = 0.
    # e = exp(s - C): ONE batched op for the whole shard (ScalarE op
    # overhead is ~1us, fewer-bigger wins), fp16 out for the PE.
    nc.scalar.activation(e_sb[:], s_sb[:],
                         mybir.ActivationFunctionType.Exp,
                         bias=shift_sb[:])
    # Pooling: accumulate e_c . x_c into PSUM [1, D] per sample. The tail
    # overlaps the next iteration's score phase.
    for s in range(SPB):
        for c in range(ncols):
            nc.tensor.matmul(
                pool_ps[s][:],
                e_sb[:, s, c:c + 1],
                xt[:, s, c, :],
                start=(c == 0),
                stop=(c == ncols - 1),
            )
    nc.sync.dma_start(out=eout[:], in_=e_sb[:])
    # Finalize the OTHER half's pooling accumulators (deferred so nothing
    # waits on this half's own matmuls): DVE copy PSUM->SBUF, DMA out.
    # Host does out = raw/Z with Z from e.
    for s in range(SPB):
        o_sb = sm.tile([1, D], _F32, name=f"o_{s}")
        nc.vector.tensor_copy(o_sb[:], ps[fin_half][s][:])
        nc.sync.dma_start(out=out[s:s + 1, :], in_=o_sb[:])


def _get_program(ncols):
    if ncols not in _CACHE:
        _CACHE[ncols] = _build_program(ncols)
    return _CACHE[ncols]


def _prep_inputs(x, flat_mask, W, v):
    """Compact to valid rows, swizzle partition-major; (in_maps, meta)."""
    x = np.ascontiguousarray(x, dtype=np.float32)
    flat_mask = np.asarray(flat_mask)
    W = np.asarray(W, dtype=np.float32)
    v = np.asarray(v, dtype=np.float32)
    # scores = x @ u + (b . v); the constant is dropped by softmax invariance.
    u = (v @ W).astype(np.float16)
    # replicated 4x along free dim for the 4-column fused product op
    u_rep = np.ascontiguousarray(
        np.broadcast_to(np.tile(u, 4), (128, 4 * D)), dtype=np.float16)

    idxs = [np.nonzero(flat_mask[b] == 1)[0] for b in range(B)]
    counts = np.array([len(ix) for ix in idxs])
    ncols = max(1, int(-(-counts.max() // 128)))
    ncap = ncols * 128

    # Masking without any device ops: padding rows are set to
    # x_pad = -(KAPPA/||u||^2) * u, so their score is exactly -KAPPA
    # (exp -> 0 in fp32) and their pooling term is e*x = 0*x_pad = 0.
    # |x_pad . u elementwise| <= KAPPA, safely inside fp16 range.
    u64 = u.astype(np.float64)
    unorm2 = float((u64 * u64).sum())
    KAPPA = 1.0e4
    degenerate = not (unorm2 > 0.0
                      and KAPPA * float(np.abs(u64).max()) / unorm2 < 6.0e4)
    if degenerate:
        x_pad = np.zeros((D,), dtype=np.float16)
    else:
        x_pad = (-(KAPPA / unorm2) * u64).astype(np.float16)

    xc = np.empty((B, ncap, D), dtype=np.float16)
    for b in range(B):
        cnt = counts[b]
        if cnt:
            xc[b, :cnt] = x[b, idxs[b]]
        xc[b, cnt:] = x_pad
    # row = col*128 + p  ->  [B, 128, ncols, D] partition-major
    xc = xc.reshape(B, ncols, 128, D).transpose(0, 2, 1, 3)

    in_maps = []
    for core in range(N_CORES):
        lo = core * SPB
        in_maps.append({
            # [128, SPB, ncols, D]
            "x": np.ascontiguousarray(xc[lo:lo + SPB].transpose(1, 0, 2, 3)),
            "u": u_rep,
        })
    meta = {"ncols": ncols, "mask_in_stt": False, "counts": counts,
            "degenerate": degenerate}
    return in_maps, meta


def kernel(x, flat_mask, W, b, v, **_unused):
    in_maps, meta = _prep_inputs(x, flat_mask, W, v)
    nc = _get_program(meta["ncols"])
    res = run_bass_kernel_spmd(nc, in_maps, core_ids=list(range(N_CORES)))
    raw = np.concatenate([res.results[i]["out"] for i in range(N_CORES)],
                         axis=0)
    z = np.concatenate(
        [res.results[i]["eout"].astype(np.float32).sum(axis=(0, 2))
         for i in range(N_CORES)], axis=0)
    out = (raw / z[:, None]).astype(np.float32)
    if (meta["counts"] == 0).any():
        # Reference semantics for an all-masked sample: uniform mean pool.
        x = np.asarray(x, dtype=np.float32)
        for bi in np.nonzero(meta["counts"] == 0)[0]:
            out[bi] = x[bi].mean(axis=0)
    if meta["degenerate"]:
        # Near-zero or pathological u = v@W: the x_pad masking trick can't
        # represent the padding rows in fp16. Tiny host fallback (never
        # triggers for randn-scale inputs).
        x = np.asarray(x, dtype=np.float32)
        u = (np.asarray(v, np.float64) @ np.asarray(W, np.float64))
        for bi in range(B):
            m = np.asarray(flat_mask[bi]) == 1
            if not m.any():
                continue
            s = x[bi, m].astype(np.float64) @ u
            w = np.exp(s - s.max())
            w /= w.sum()
            out[bi] = (w[:, None] * x[bi, m]).sum(0).astype(np.float32)
    return out


# revision 16
# speedup vs baseline: 1.0873x; 1.0873x over previous
"""Trainium2 Bass kernel for masked attention-pooling (DmasifAttentionModule).

Reference computation (per sample b):
    proj   = x @ W.T + b                  # [N, D]
    scores = proj @ v                     # [N]
    scores = where(mask, scores, -1e9)
    w      = softmax(scores)              # [N]
    out    = w @ x                        # [D]

Optimizations (exact up to fp reassociation unless noted):
  1. scores = x @ (W.T @ v) + (b . v); softmax is shift-invariant, so the
     (b . v) constant drops out and the 34-GFLOP projection collapses to a
     matvec against u = v @ W (host-computed, 512 floats).
  2. Masked rows get softmax weight exactly 0, so only the ~50% valid rows
     participate. The host compacts each sample to its valid rows (padded
     to a common column count with zero rows) and streams only those.
  3. x and u ship as fp16: halves HBM traffic (the binding resource), runs
     the pooling matmul at full PE rate (fp32 = 4 passes) and keeps DVE
     elementwise ops in 2x_1p mode. Score accumulation stays fp32.
  4. The compacted shard is host-swizzled to partition-major
     [128, SPB, ncols, D] so the whole 4.45 MiB arrives as ONE dma_start
     with 17.4 KiB contiguous per partition (meas. 295 GB/s vs 245 for
     per-tile strided transfers). Double-buffered across For_i iterations.
  5. Scores: a DVE free-dim reduce only has a 1x uop (694 ns/[128,512]
     column) while plain tensor_tensor runs 2x (438 ns), so columns are
     split: ~half fused on DVE (scalar_tensor_tensor w/ accum), the rest
     as DVE 2x products + ScalarE Copy-with-accum reduce (872 ns, ScalarE
     is otherwise idle). Masking is a single posthoc [128,ncols]
     tensor_add of -3e8 per masked column (no fp16-range contortions).
  6. exp: one batched ScalarE activation per sample (bias = -C shift),
     fp16 out. No accum: Z is recovered on host from the e tensor itself
     (8.7 KiB DMA per core), so numerator and denominator use bit-identical
     weights.
  7. Pooling: TensorE matvec accumulation into PSUM [1,512] per sample
     (lhsT = e column [128,1] fp16, rhs = x column [128,512] fp16,
     216 ns each); ScalarE copies PSUM out, host divides by Z.

Per-core budgets at ncols=17 (8 cores, 2 samples each, data-parallel):
DMA ~15.1 us, DVE ~18.7 us, ACT ~19 us, PE ~8 us -> ~20 us/iter steady.
"""

import os
import sys

import numpy as np

for _p in ("/opt/trn_rl_repo", "/root/.axon_site/_ro/trn_rl_repo"):
    if os.path.isdir(_p) and _p not in sys.path:
        sys.path.append(_p)

import concourse.bacc as bacc
import concourse.tile as tile
from concourse import mybir
from concourse.bass_utils import run_bass_kernel_spmd

B, N, D = 16, 4096, 512
N_CORES = 8
SPB = B // N_CORES          # samples per core
C_SHIFT = 24.0              # constant exp-range shift (softmax-invariant)
MASKED_INIT = -3.0e8        # masked scores -> exp underflows to exactly 0
ACT_COLS = 8                # score columns per sample reduced on ScalarE

_F32 = mybir.dt.float32
_F16 = mybir.dt.float16
_CACHE = {}


def _build_program(ncols, loop_n=None, act_cols=None, mask_in_stt=None):
    """Program for samples compacted to `ncols` columns of 128 rows each.

    loop_n wraps the computation in a HW For_i loop (timing only).
    mask_in_stt is accepted for test.py compatibility and ignored."""
    if act_cols is None:
        act_cols = ACT_COLS
    na = min(act_cols, max(0, ncols - 1))   # ScalarE-reduced cols per sample
    # Contiguous runs of <=4 so the DVE product op covers a whole run
    # (one [128, 4*512] 2x tensor_tensor = 1224 ns vs 4x438 split).
    quads = [(c0, min(4, na - c0)) for c0 in range(0, na, 4)]

    nc = bacc.Bacc("TRN2", target_bir_lowering=False, debug=False)
    x = nc.dram_tensor("x", [128, SPB * ncols * D], _F16,
                       kind="ExternalInput").ap()
    u = nc.dram_tensor("u", [128, 4 * D], _F16, kind="ExternalInput").ap()
    out = nc.dram_tensor("out", [SPB, D], _F32, kind="ExternalOutput").ap()
    eout = nc.dram_tensor("eout", [128, SPB * ncols], _F16,
                          kind="ExternalOutput").ap()

    with tile.TileContext(nc) as tc:
        with (
            tc.tile_pool(name="xp", bufs=2) as xp,
            tc.tile_pool(name="singles", bufs=1) as sg,
            tc.tile_pool(name="prod", bufs=4) as prp,
            tc.tile_pool(name="scratch", bufs=2) as scr,
            tc.tile_pool(name="smalls", bufs=2) as sm,
            tc.tile_pool(name="ps", bufs=1, space="PSUM") as psp,
        ):
            ones_sb = sg.tile([128, 1], _F32)
            nc.vector.memset(ones_sb[:], 1.0)
            shift_sb = sg.tile([128, 1], _F32)
            nc.vector.memset(shift_sb[:], -C_SHIFT)
            warm = sg.tile([128, 1], _F32)
            # Pull the exp table-set load (~2.7us) to t=0, under the DMAs.
            nc.scalar.activation(warm[:], ones_sb[:],
                                 mybir.ActivationFunctionType.Exp)

            u_sb = sg.tile([128, 4 * D], _F16)  # u replicated 4x along free
            nc.sync.dma_start(out=u_sb[:], in_=u[:])

            # PSUM pooling accumulators for both unrolled halves, so each
            # half's finalize copies can be deferred into the other half
            # (by which time the pooling matmuls are long done -> no stall).
            ps = [{s: psp.tile([1, D], _F32, name=f"ps_{h}_{s}")
                   for s in range(SPB)} for h in range(2)]

            ctx = (nc, xp, prp, scr, sm, x, out, eout, u_sb,
                   shift_sb, ncols, quads, na, ps)

            if loop_n is not None:
                # For_i is a HW loop over a STATIC body: tile-pool rotation
                # only happens across emit calls, so double buffering needs
                # the body unrolled x2 (iteration i+1's DMA lands in the
                # other buffer and overlaps iteration i's compute).
                assert loop_n % 2 == 0, loop_n
                with tc.For_i(0, loop_n // 2, 1) as _i:
                    _emit_iteration(*ctx, half=0, fin_half=1)
                    _emit_iteration(*ctx, half=1, fin_half=0)
            else:
                _emit_iteration(*ctx, half=0, fin_half=0)

    nc.compile()
    return nc


def _emit_iteration(nc, xp, prp, scr, sm, x, out, eout, u_sb,
                    shift_sb, ncols, quads, na, ps, half, fin_half):
    # One DMA for the whole shard; double-buffered across the two unrolled
    # halves so the transfer of iteration i+1 overlaps compute of i.
    # Everything is kept as FLAT 2D tiles/APs: 3D-sliced operands measurably
    # slow DVE ops (~70-170 ns/op of AP overhead).
    xt = xp.tile([128, SPB * ncols * D], _F16, name="xt")
    nc.sync.dma_start(out=xt[:], in_=x[:])
    # Per-emit score/e tiles (rotate with the unrolled halves) so the two
    # in-flight iterations never alias.
    s_sb = xp.tile([128, SPB * ncols], _F32, name="s_sb")
    e_sb = xp.tile([128, SPB * ncols], _F16, name="e_sb")
    pool_ps = ps[half]

    def xcol(s, c, w=1):
        o = (s * ncols + c) * D
        return xt[:, o:o + w * D]

    for s in range(SPB):
        # ScalarE-routed columns first: one DVE 2x product op per <=4-col
        # run, ScalarE Copy-accum reduces stream behind it per column.
        for c0, cw in quads:
            prod = prp.tile([128, cw * D], _F16, name=f"prod{c0}")
            nc.vector.tensor_tensor(
                out=prod[:], in0=xcol(s, c0, cw),
                in1=u_sb[:, 0:cw * D], op=mybir.AluOpType.mult)
            for j in range(cw):
                i = s * ncols + c0 + j
                dump32 = scr.tile([128, D], _F32, name="dump32")
                nc.scalar.activation(
                    dump32[:], prod[:, j * D:(j + 1) * D],
                    mybir.ActivationFunctionType.Copy,
                    accum_out=s_sb[:, i:i + 1])
        # Remaining columns fused on DVE (1x scalar_tensor_tensor w/ accum).
        for c in range(na, ncols):
            i = s * ncols + c
            dump = scr.tile([128, D], _F16, name="dump")
            nc.vector.scalar_tensor_tensor(
                out=dump[:], in0=xcol(s, c),
                scalar=0.0, in1=u_sb[:, 0:D],
                op0=mybir.AluOpType.add, op1=mybir.AluOpType.mult,
                accum_out=s_sb[:, i:i + 1])
        # Masking needs no ops: the host writes padding rows as
        # x_pad = -kappa*u/||u||^2, so their score is exactly -kappa
        # (exp -> 0) and their pooling contribution is e*x = 0*x_pad = 0.
        # e = exp(s - C), one batched op per sample, fp16 out for the PE.
        nc.scalar.activation(e_sb[:, s * ncols:(s + 1) * ncols],
                             s_sb[:, s * ncols:(s + 1) * ncols],
                             mybir.ActivationFunctionType.Exp,
                             bias=shift_sb[:])
        # Pooling: accumulate e_c . x_c into PSUM [1, D]; overlaps the next
        # sample's (and iteration's) score work.
        for c in range(ncols):
            i = s * ncols + c
            nc.tensor.matmul(
                pool_ps[s][:],
                e_sb[:, i:i + 1],
                xcol(s, c),
                start=(c == 0),
                stop=(c == ncols - 1),
            )
    nc.sync.dma_start(out=eout[:], in_=e_sb[:])
    # Finalize the OTHER half's pooling accumulators (deferred so nothing
    # waits on this half's own matmuls): DVE copy PSUM->SBUF, DMA out.
    # Host does out = raw/Z with Z from e.
    for s in range(SPB):
        o_sb = sm.tile([1, D], _F32, name=f"o_{s}")
        nc.vector.tensor_copy(o_sb[:], ps[fin_half][s][:])
        nc.sync.dma_start(out=out[s:s + 1, :], in_=o_sb[:])


def _get_program(ncols):
    if ncols not in _CACHE:
        _CACHE[ncols] = _build_program(ncols)
    return _CACHE[ncols]


def _prep_inputs(x, flat_mask, W, v):
    """Compact to valid rows, swizzle partition-major; (in_maps, meta)."""
    x = np.ascontiguousarray(x, dtype=np.float32)
    flat_mask = np.asarray(flat_mask)
    W = np.asarray(W, dtype=np.float32)
    v = np.asarray(v, dtype=np.float32)
    # scores = x @ u + (b . v); the constant is dropped by softmax invariance.
    u = (v @ W).astype(np.float16)
    # replicated 4x along free dim for the 4-column fused product op
    u_rep = np.ascontiguousarray(
        np.broadcast_to(np.tile(u, 4), (128, 4 * D)), dtype=np.float16)

    idxs = [np.nonzero(flat_mask[b] == 1)[0] for b in range(B)]
    counts = np.array([len(ix) for ix in idxs])
    ncols = max(1, int(-(-counts.max() // 128)))
    ncap = ncols * 128

    # Masking without any device ops: padding rows are set to
    # x_pad = -(KAPPA/||u||^2) * u, so their score is exactly -KAPPA
    # (exp -> 0 in fp32) and their pooling term is e*x = 0*x_pad = 0.
    # |x_pad . u elementwise| <= KAPPA, safely inside fp16 range.
    u64 = u.astype(np.float64)
    unorm2 = float((u64 * u64).sum())
    KAPPA = 1.0e4
    degenerate = not (unorm2 > 0.0
                      and KAPPA * float(np.abs(u64).max()) / unorm2 < 6.0e4)
    if degenerate:
        x_pad = np.zeros((D,), dtype=np.float16)
    else:
        x_pad = (-(KAPPA / unorm2) * u64).astype(np.float16)

    xc = np.empty((B, ncap, D), dtype=np.float16)
    for b in range(B):
        cnt = counts[b]
        if cnt:
            xc[b, :cnt] = x[b, idxs[b]]
        xc[b, cnt:] = x_pad
    # row = col*128 + p  ->  [B, 128, ncols, D] partition-major
    xc = xc.reshape(B, ncols, 128, D).transpose(0, 2, 1, 3)

    in_maps = []
    for core in range(N_CORES):
        lo = core * SPB
        in_maps.append({
            # [128, SPB*ncols*D] flat partition-major
            "x": np.ascontiguousarray(
                xc[lo:lo + SPB].transpose(1, 0, 2, 3)).reshape(128, -1),
            "u": u_rep,
        })
    meta = {"ncols": ncols, "mask_in_stt": False, "counts": counts,
            "degenerate": degenerate}
    return in_maps, meta


def kernel(x, flat_mask, W, b, v, **_unused):
    in_maps, meta = _prep_inputs(x, flat_mask, W, v)
    nc = _get_program(meta["ncols"])
    res = run_bass_kernel_spmd(nc, in_maps, core_ids=list(range(N_CORES)))
    raw = np.concatenate([res.results[i]["out"] for i in range(N_CORES)],
                         axis=0)
    nct = in_maps[0]["x"].shape[1] // (SPB * D)
    z = np.concatenate(
        [res.results[i]["eout"].reshape(128, SPB, nct)
         .astype(np.float32).sum(axis=(0, 2))
         for i in range(N_CORES)], axis=0)
    out = (raw / z[:, None]).astype(np.float32)
    if (meta["counts"] == 0).any():
        # Reference semantics for an all-masked sample: uniform mean pool.
        x = np.asarray(x, dtype=np.float32)
        for bi in np.nonzero(meta["counts"] == 0)[0]:
            out[bi] = x[bi].mean(axis=0)
    if meta["degenerate"]:
        # Near-zero or pathological u = v@W: the x_pad masking trick can't
        # represent the padding rows in fp16. Tiny host fallback (never
        # triggers for randn-scale inputs).
        x = np.asarray(x, dtype=np.float32)
        u = (np.asarray(v, np.float64) @ np.asarray(W, np.float64))
        for bi in range(B):
            m = np.asarray(flat_mask[bi]) == 1
            if not m.any():
                continue
            s = x[bi, m].astype(np.float64) @ u
            w = np.exp(s - s.max())
            w /= w.sum()
            out[bi] = (w[:, None] * x[bi, m]).sum(0).astype(np.float32)
    return out


# revision 18
# speedup vs baseline: 1.0998x; 1.0115x over previous
"""Trainium2 Bass kernel for masked attention-pooling (DmasifAttentionModule).

Reference computation (per sample b):
    proj   = x @ W.T + b                  # [N, D]
    scores = proj @ v                     # [N]
    scores = where(mask, scores, -1e9)
    w      = softmax(scores)              # [N]
    out    = w @ x                        # [D]

Optimizations (exact up to fp reassociation unless noted):
  1. scores = x @ (W.T @ v) + (b . v); softmax is shift-invariant, so the
     (b . v) constant drops out and the 34-GFLOP projection collapses to a
     matvec against u = v @ W (host-computed, 512 floats).
  2. Masked rows get softmax weight exactly 0, so only the ~50% valid rows
     participate. The host compacts each sample to its valid rows (padded
     to a common column count with zero rows) and streams only those.
  3. x and u ship as fp16: halves HBM traffic (the binding resource), runs
     the pooling matmul at full PE rate (fp32 = 4 passes) and keeps DVE
     elementwise ops in 2x_1p mode. Score accumulation stays fp32.
  4. The compacted shard is host-swizzled to partition-major
     [128, SPB, ncols, D] so the whole 4.45 MiB arrives as ONE dma_start
     with 17.4 KiB contiguous per partition (meas. 295 GB/s vs 245 for
     per-tile strided transfers). Double-buffered across For_i iterations.
  5. Scores: a DVE free-dim reduce only has a 1x uop (694 ns/[128,512]
     column) while plain tensor_tensor runs 2x (438 ns), so columns are
     split: ~half fused on DVE (scalar_tensor_tensor w/ accum), the rest
     as DVE 2x products + ScalarE Copy-with-accum reduce (872 ns, ScalarE
     is otherwise idle). Masking is a single posthoc [128,ncols]
     tensor_add of -3e8 per masked column (no fp16-range contortions).
  6. exp: one batched ScalarE activation per sample (bias = -C shift),
     fp16 out. No accum: Z is recovered on host from the e tensor itself
     (8.7 KiB DMA per core), so numerator and denominator use bit-identical
     weights.
  7. Pooling: TensorE matvec accumulation into PSUM [1,512] per sample
     (lhsT = e column [128,1] fp16, rhs = x column [128,512] fp16,
     216 ns each); ScalarE copies PSUM out, host divides by Z.

Per-core budgets at ncols=17 (8 cores, 2 samples each, data-parallel):
DMA ~15.1 us, DVE ~18.7 us, ACT ~19 us, PE ~8 us -> ~20 us/iter steady.
"""

import os
import sys

import numpy as np

for _p in ("/opt/trn_rl_repo", "/root/.axon_site/_ro/trn_rl_repo"):
    if os.path.isdir(_p) and _p not in sys.path:
        sys.path.append(_p)

import concourse.bacc as bacc
import concourse.tile as tile
from concourse import mybir
from concourse.bass_utils import run_bass_kernel_spmd

B, N, D = 16, 4096, 512
N_CORES = 8
SPB = B // N_CORES          # samples per core
C_SHIFT = 24.0              # constant exp-range shift (softmax-invariant)
MASKED_INIT = -3.0e8        # masked scores -> exp underflows to exactly 0
ACT_COLS = 8                # score columns per sample reduced on ScalarE

_F32 = mybir.dt.float32
_F16 = mybir.dt.float16
_CACHE = {}


def _build_program(ncols, loop_n=None, act_cols=None, mask_in_stt=None):
    """Program for samples compacted to `ncols` columns of 128 rows each.

    loop_n wraps the computation in a HW For_i loop (timing only).
    mask_in_stt is accepted for test.py compatibility and ignored."""
    if act_cols is None:
        act_cols = ACT_COLS
    na = min(act_cols, max(0, ncols - 1))   # ScalarE-reduced cols per sample
    # Contiguous runs of <=4 so the DVE product op covers a whole run
    # (one [128, 4*512] 2x tensor_tensor = 1224 ns vs 4x438 split).
    quads = [(c0, min(4, na - c0)) for c0 in range(0, na, 4)]

    nc = bacc.Bacc("TRN2", target_bir_lowering=False, debug=False)
    x = nc.dram_tensor("x", [128, SPB * ncols * D], _F16,
                       kind="ExternalInput").ap()
    u = nc.dram_tensor("u", [128, 4 * D], _F16, kind="ExternalInput").ap()
    out = nc.dram_tensor("out", [SPB, D], _F32, kind="ExternalOutput").ap()
    eout = nc.dram_tensor("eout", [128, SPB * ncols], _F16,
                          kind="ExternalOutput").ap()

    with tile.TileContext(nc) as tc:
        with (
            tc.tile_pool(name="xp", bufs=2) as xp,
            tc.tile_pool(name="singles", bufs=1) as sg,
            tc.tile_pool(name="prod", bufs=4) as prp,
            tc.tile_pool(name="scratch", bufs=2) as scr,
            tc.tile_pool(name="smalls", bufs=2) as sm,
            tc.tile_pool(name="ps", bufs=1, space="PSUM") as psp,
        ):
            ones_sb = sg.tile([128, 1], _F32)
            nc.vector.memset(ones_sb[:], 1.0)
            shift_sb = sg.tile([128, 1], _F32)
            nc.vector.memset(shift_sb[:], -C_SHIFT)
            warm = sg.tile([128, 1], _F32)
            # Pull the exp table-set load (~2.7us) to t=0, under the DMAs.
            nc.scalar.activation(warm[:], ones_sb[:],
                                 mybir.ActivationFunctionType.Exp)

            u_sb = sg.tile([128, 4 * D], _F16)  # u replicated 4x along free
            nc.sync.dma_start(out=u_sb[:], in_=u[:])

            # PSUM pooling accumulators for both unrolled halves, so each
            # half's finalize copies can be deferred into the other half
            # (by which time the pooling matmuls are long done -> no stall).
            ps = [{s: psp.tile([1, D], _F32, name=f"ps_{h}_{s}")
                   for s in range(SPB)} for h in range(2)]
            for h in range(2):
                for s in range(SPB):
                    # The loop body finalizes each half's PSUM one For_i
                    # body late; initialize so the first read is defined.
                    nc.vector.memset(ps[h][s][:], 0.0)

            ctx = (nc, xp, prp, scr, sm, x, out, eout, u_sb,
                   shift_sb, ncols, quads, na, ps)

            if loop_n is not None:
                # For_i is a HW loop over a STATIC body: tile-pool rotation
                # only happens across emit calls, so double buffering needs
                # the body unrolled x2 (iteration i+1's DMA lands in the
                # other buffer and overlaps iteration i's compute).
                assert loop_n % 2 == 0, loop_n
                with tc.For_i(0, loop_n // 2, 1) as _i:
                    _emit_iteration(*ctx, half=0, fin_top=True)
                    _emit_iteration(*ctx, half=1, fin_top=True)
            else:
                _emit_iteration(*ctx, half=0, fin_top=False)

    nc.compile()
    return nc


def _emit_iteration(nc, xp, prp, scr, sm, x, out, eout, u_sb,
                    shift_sb, ncols, quads, na, ps, half, fin_top):
    # DMA-ring discipline: the big x transfer is the ONLY nc.sync DMA, so
    # its HWDGE ring never stalls on a semaphore of a small output DMA and
    # iteration i+1's transfer genuinely overlaps iteration i's compute.
    # All small output DMAs ride the other ring (nc.scalar / ACT queue) at
    # points where their dependencies are already retired.
    def _finalize():
        # Finalize THIS half's PSUM accumulators from one For_i body ago
        # (ancient -> zero stall): DVE copy PSUM->SBUF, out DMA on the
        # scalar ring. Host does out = raw/Z with Z from e.
        for s in range(SPB):
            o_sb = sm.tile([1, D], _F32, name=f"o_{s}")
            nc.vector.tensor_copy(o_sb[:], ps[half][s][:])
            nc.scalar.dma_start(out=out[s:s + 1, :], in_=o_sb[:])

    if fin_top:
        _finalize()
    # One DMA for the whole shard; double-buffered across the two unrolled
    # halves so the transfer of iteration i+1 overlaps compute of i.
    # Everything is kept as FLAT 2D tiles/APs: 3D-sliced operands measurably
    # slow DVE ops (~70-170 ns/op of AP overhead).
    xt = xp.tile([128, SPB * ncols * D], _F16, name="xt")
    nc.sync.dma_start(out=xt[:], in_=x[:])
    # Per-emit score/e tiles (rotate with the unrolled halves) so the two
    # in-flight iterations never alias.
    s_sb = xp.tile([128, SPB * ncols], _F32, name="s_sb")
    e_sb = xp.tile([128, SPB * ncols], _F16, name="e_sb")
    pool_ps = ps[half]

    def xcol(s, c, w=1):
        o = (s * ncols + c) * D
        return xt[:, o:o + w * D]

    for s in range(SPB):
        # ScalarE-routed columns first: one DVE 2x product op per <=4-col
        # run, ScalarE Copy-accum reduces stream behind it per column.
        for c0, cw in quads:
            prod = prp.tile([128, cw * D], _F16, name=f"prod{c0}")
            nc.vector.tensor_tensor(
                out=prod[:], in0=xcol(s, c0, cw),
                in1=u_sb[:, 0:cw * D], op=mybir.AluOpType.mult)
            for j in range(cw):
                i = s * ncols + c0 + j
                dump32 = scr.tile([128, D], _F32, name="dump32")
                nc.scalar.activation(
                    dump32[:], prod[:, j * D:(j + 1) * D],
                    mybir.ActivationFunctionType.Copy,
                    accum_out=s_sb[:, i:i + 1])
        # Remaining columns fused on DVE (1x scalar_tensor_tensor w/ accum).
        for c in range(na, ncols):
            i = s * ncols + c
            dump = scr.tile([128, D], _F16, name="dump")
            nc.vector.scalar_tensor_tensor(
                out=dump[:], in0=xcol(s, c),
                scalar=0.0, in1=u_sb[:, 0:D],
                op0=mybir.AluOpType.add, op1=mybir.AluOpType.mult,
                accum_out=s_sb[:, i:i + 1])
        # Masking needs no ops: the host writes padding rows as
        # x_pad = -kappa*u/||u||^2, so their score is exactly -kappa
        # (exp -> 0) and their pooling contribution is e*x = 0*x_pad = 0.
        # e = exp(s - C), one batched op per sample, fp16 out for the PE.
        nc.scalar.activation(e_sb[:, s * ncols:(s + 1) * ncols],
                             s_sb[:, s * ncols:(s + 1) * ncols],
                             mybir.ActivationFunctionType.Exp,
                             bias=shift_sb[:])
        # Pooling: accumulate e_c . x_c into PSUM [1, D]; overlaps the next
        # sample's (and iteration's) score work.
        for c in range(ncols):
            i = s * ncols + c
            nc.tensor.matmul(
                pool_ps[s][:],
                e_sb[:, i:i + 1],
                xcol(s, c),
                start=(c == 0),
                stop=(c == ncols - 1),
            )
    nc.scalar.dma_start(out=eout[:], in_=e_sb[:])
    if not fin_top:
        _finalize()


def _get_program(ncols):
    if ncols not in _CACHE:
        _CACHE[ncols] = _build_program(ncols)
    return _CACHE[ncols]


def _prep_inputs(x, flat_mask, W, v):
    """Compact to valid rows, swizzle partition-major; (in_maps, meta)."""
    x = np.ascontiguousarray(x, dtype=np.float32)
    flat_mask = np.asarray(flat_mask)
    W = np.asarray(W, dtype=np.float32)
    v = np.asarray(v, dtype=np.float32)
    # scores = x @ u + (b . v); the constant is dropped by softmax invariance.
    u = (v @ W).astype(np.float16)
    # replicated 4x along free dim for the 4-column fused product op
    u_rep = np.ascontiguousarray(
        np.broadcast_to(np.tile(u, 4), (128, 4 * D)), dtype=np.float16)

    idxs = [np.nonzero(flat_mask[b] == 1)[0] for b in range(B)]
    counts = np.array([len(ix) for ix in idxs])
    ncols = max(1, int(-(-counts.max() // 128)))
    ncap = ncols * 128

    # Masking without any device ops: padding rows are set to
    # x_pad = -(KAPPA/||u||^2) * u, so their score is exactly -KAPPA
    # (exp -> 0 in fp32) and their pooling term is e*x = 0*x_pad = 0.
    # |x_pad . u elementwise| <= KAPPA, safely inside fp16 range.
    u64 = u.astype(np.float64)
    unorm2 = float((u64 * u64).sum())
    KAPPA = 1.0e4
    degenerate = not (unorm2 > 0.0
                      and KAPPA * float(np.abs(u64).max()) / unorm2 < 6.0e4)
    if degenerate:
        x_pad = np.zeros((D,), dtype=np.float16)
    else:
        x_pad = (-(KAPPA / unorm2) * u64).astype(np.float16)

    xc = np.empty((B, ncap, D), dtype=np.float16)
    for b in range(B):
        cnt = counts[b]
        if cnt:
            xc[b, :cnt] = x[b, idxs[b]]
        xc[b, cnt:] = x_pad
    # row = col*128 + p  ->  [B, 128, ncols, D] partition-major
    xc = xc.reshape(B, ncols, 128, D).transpose(0, 2, 1, 3)

    in_maps = []
    for core in range(N_CORES):
        lo = core * SPB
        in_maps.append({
            # [128, SPB*ncols*D] flat partition-major
            "x": np.ascontiguousarray(
                xc[lo:lo + SPB].transpose(1, 0, 2, 3)).reshape(128, -1),
            "u": u_rep,
        })
    meta = {"ncols": ncols, "mask_in_stt": False, "counts": counts,
            "degenerate": degenerate}
    return in_maps, meta


def kernel(x, flat_mask, W, b, v, **_unused):
    in_maps, meta = _prep_inputs(x, flat_mask, W, v)
    nc = _get_program(meta["ncols"])
    res = run_bass_kernel_spmd(nc, in_maps, core_ids=list(range(N_CORES)))
    raw = np.concatenate([res.results[i]["out"] for i in range(N_CORES)],
                         axis=0)
    nct = in_maps[0]["x"].shape[1] // (SPB * D)
    z = np.concatenate(
        [res.results[i]["eout"].reshape(128, SPB, nct)
         .astype(np.float32).sum(axis=(0, 2))
         for i in range(N_CORES)], axis=0)
    out = (raw / z[:, None]).astype(np.float32)
    if (meta["counts"] == 0).any():
        # Reference semantics for an all-masked sample: uniform mean pool.
        x = np.asarray(x, dtype=np.float32)
        for bi in np.nonzero(meta["counts"] == 0)[0]:
            out[bi] = x[bi].mean(axis=0)
    if meta["degenerate"]:
        # Near-zero or pathological u = v@W: the x_pad masking trick can't
        # represent the padding rows in fp16. Tiny host fallback (never
        # triggers for randn-scale inputs).
        x = np.asarray(x, dtype=np.float32)
        u = (np.asarray(v, np.float64) @ np.asarray(W, np.float64))
        for bi in range(B):
            m = np.asarray(flat_mask[bi]) == 1
            if not m.any():
                continue
            s = x[bi, m].astype(np.float64) @ u
            w = np.exp(s - s.max())
            w /= w.sum()
            out[bi] = (w[:, None] * x[bi, m]).sum(0).astype(np.float32)
    return out


# revision 19
# speedup vs baseline: 1.5427x; 1.4027x over previous
"""Trainium2 Bass kernel for masked attention-pooling (DmasifAttentionModule).

Reference computation (per sample b):
    proj   = x @ W.T + b                  # [N, D]
    scores = proj @ v                     # [N]
    scores = where(mask, scores, -1e9)
    w      = softmax(scores)              # [N]
    out    = w @ x                        # [D]

Optimizations (exact up to fp reassociation unless noted):
  1. scores = x @ (W.T @ v) + (b . v); softmax is shift-invariant, so the
     (b . v) constant drops out and the 34-GFLOP projection collapses to a
     matvec against u = v @ W (host-computed, 512 floats).
  2. Masked rows get softmax weight exactly 0, so only the ~50% valid rows
     participate. The host compacts each sample to its valid rows (padded
     to a common column count with zero rows) and streams only those.
  3. x and u ship as fp16: halves HBM traffic (the binding resource), runs
     the pooling matmul at full PE rate (fp32 = 4 passes) and keeps DVE
     elementwise ops in 2x_1p mode. Score accumulation stays fp32.
  4. The compacted shard is host-swizzled to partition-major
     [128, SPB, ncols, D] so the whole 4.45 MiB arrives as ONE dma_start
     with 17.4 KiB contiguous per partition (meas. 295 GB/s vs 245 for
     per-tile strided transfers). Double-buffered across For_i iterations.
  5. Scores: a DVE free-dim reduce only has a 1x uop (694 ns/[128,512]
     column) while plain tensor_tensor runs 2x (438 ns), so columns are
     split: ~half fused on DVE (scalar_tensor_tensor w/ accum), the rest
     as DVE 2x products + ScalarE Copy-with-accum reduce (872 ns, ScalarE
     is otherwise idle). Masking is a single posthoc [128,ncols]
     tensor_add of -3e8 per masked column (no fp16-range contortions).
  6. exp: one batched ScalarE activation per sample (bias = -C shift),
     fp16 out. No accum: Z is recovered on host from the e tensor itself
     (8.7 KiB DMA per core), so numerator and denominator use bit-identical
     weights.
  7. Pooling: TensorE matvec accumulation into PSUM [1,512] per sample
     (lhsT = e column [128,1] fp16, rhs = x column [128,512] fp16,
     216 ns each); ScalarE copies PSUM out, host divides by Z.

Per-core budgets at ncols=17 (8 cores, 2 samples each, data-parallel):
DMA ~15.1 us, DVE ~18.7 us, ACT ~19 us, PE ~8 us -> ~20 us/iter steady.
"""

import os
import sys

import numpy as np

for _p in ("/opt/trn_rl_repo", "/root/.axon_site/_ro/trn_rl_repo"):
    if os.path.isdir(_p) and _p not in sys.path:
        sys.path.append(_p)

import concourse.bacc as bacc
import concourse.tile as tile
from concourse import mybir
from concourse.bass_utils import run_bass_kernel_spmd

B, N, D = 16, 4096, 512
N_CORES = 8
SPB = B // N_CORES          # samples per core
C_SHIFT = 24.0              # constant exp-range shift (softmax-invariant)
MASKED_INIT = -3.0e8        # masked scores -> exp underflows to exactly 0
ACT_COLS = 8                # score columns per sample reduced on ScalarE
UNROLL = 16                 # For_i body unroll (see _build_program)

_F32 = mybir.dt.float32
_F16 = mybir.dt.float16
_CACHE = {}


def _build_program(ncols, loop_n=None, act_cols=None, mask_in_stt=None):
    """Program for samples compacted to `ncols` columns of 128 rows each.

    loop_n wraps the computation in a HW For_i loop (timing only).
    mask_in_stt is accepted for test.py compatibility and ignored."""
    if act_cols is None:
        act_cols = ACT_COLS
    na = min(act_cols, max(0, ncols - 1))   # ScalarE-reduced cols per sample
    # Contiguous runs of <=4 so the DVE product op covers a whole run
    # (one [128, 4*512] 2x tensor_tensor = 1224 ns vs 4x438 split).
    quads = [(c0, min(4, na - c0)) for c0 in range(0, na, 4)]

    nc = bacc.Bacc("TRN2", target_bir_lowering=False, debug=False)
    x = nc.dram_tensor("x", [128, SPB * ncols * D], _F16,
                       kind="ExternalInput").ap()
    u = nc.dram_tensor("u", [128, 4 * D], _F16, kind="ExternalInput").ap()
    out = nc.dram_tensor("out", [SPB, D], _F32, kind="ExternalOutput").ap()
    eout = nc.dram_tensor("eout", [128, SPB * ncols], _F16,
                          kind="ExternalOutput").ap()

    with tile.TileContext(nc) as tc:
        with (
            tc.tile_pool(name="xp", bufs=3) as xp,
            tc.tile_pool(name="singles", bufs=1) as sg,
            tc.tile_pool(name="prod", bufs=4) as prp,
            tc.tile_pool(name="scratch", bufs=2) as scr,
            tc.tile_pool(name="smalls", bufs=2) as sm,
            tc.tile_pool(name="ps", bufs=1, space="PSUM") as psp,
        ):
            ones_sb = sg.tile([128, 1], _F32)
            nc.vector.memset(ones_sb[:], 1.0)
            shift_sb = sg.tile([128, 1], _F32)
            nc.vector.memset(shift_sb[:], -C_SHIFT)
            warm = sg.tile([128, 1], _F32)
            # Pull the exp table-set load (~2.7us) to t=0, under the DMAs.
            nc.scalar.activation(warm[:], ones_sb[:],
                                 mybir.ActivationFunctionType.Exp)

            u_sb = sg.tile([128, 4 * D], _F16)  # u replicated 4x along free
            nc.sync.dma_start(out=u_sb[:], in_=u[:])

            # PSUM pooling accumulators for both unrolled halves, so each
            # half's finalize copies can be deferred into the other half
            # (by which time the pooling matmuls are long done -> no stall).
            ps = [{s: psp.tile([1, D], _F32, name=f"ps_{h}_{s}")
                   for s in range(SPB)} for h in range(2)]
            for h in range(2):
                for s in range(SPB):
                    # The loop body finalizes each half's PSUM one For_i
                    # body late; initialize so the first read is defined.
                    nc.vector.memset(ps[h][s][:], 0.0)

            ctx = (nc, xp, prp, scr, sm, x, out, eout, u_sb,
                   shift_sb, ncols, quads, na, ps)

            if loop_n is not None:
                # For_i is a HW loop over a STATIC body, and the back-edge
                # acts as a scheduling barrier: tile-pool rotation (and so
                # DMA/compute overlap) only happens across emit calls inside
                # ONE body. Unrolling x16 (with triple-buffered x tiles)
                # amortizes the exposed leading DMA to ~1/16 and lets the
                # body pipeline internally; measured 25.3us (x2) ->
                # 17.9us (x16+bufs=3) per iteration.
                assert loop_n % UNROLL == 0, loop_n
                with tc.For_i(0, loop_n // UNROLL, 1) as _i:
                    for k in range(UNROLL):
                        _emit_iteration(*ctx, half=k % 2, fin_top=True)
            else:
                _emit_iteration(*ctx, half=0, fin_top=False)

    nc.compile()
    return nc


def _emit_iteration(nc, xp, prp, scr, sm, x, out, eout, u_sb,
                    shift_sb, ncols, quads, na, ps, half, fin_top):
    # DMA-ring discipline: the big x transfer is the ONLY nc.sync DMA, so
    # its HWDGE ring never stalls on a semaphore of a small output DMA and
    # iteration i+1's transfer genuinely overlaps iteration i's compute.
    # All small output DMAs ride the other ring (nc.scalar / ACT queue) at
    # points where their dependencies are already retired.
    def _finalize():
        # Finalize THIS half's PSUM accumulators from one For_i body ago
        # (ancient -> zero stall): DVE copy PSUM->SBUF, out DMA on the
        # scalar ring. Host does out = raw/Z with Z from e.
        for s in range(SPB):
            o_sb = sm.tile([1, D], _F32, name=f"o_{s}")
            nc.vector.tensor_copy(o_sb[:], ps[half][s][:])
            nc.scalar.dma_start(out=out[s:s + 1, :], in_=o_sb[:])

    if fin_top:
        _finalize()
    # One DMA for the whole shard; double-buffered across the two unrolled
    # halves so the transfer of iteration i+1 overlaps compute of i.
    # Everything is kept as FLAT 2D tiles/APs: 3D-sliced operands measurably
    # slow DVE ops (~70-170 ns/op of AP overhead).
    xt = xp.tile([128, SPB * ncols * D], _F16, name="xt")
    nc.sync.dma_start(out=xt[:], in_=x[:])
    # Per-emit score/e tiles (rotate with the unrolled halves) so the two
    # in-flight iterations never alias.
    s_sb = xp.tile([128, SPB * ncols], _F32, name="s_sb")
    e_sb = xp.tile([128, SPB * ncols], _F16, name="e_sb")
    pool_ps = ps[half]

    def xcol(s, c, w=1):
        o = (s * ncols + c) * D
        return xt[:, o:o + w * D]

    for s in range(SPB):
        # ScalarE-routed columns first: one DVE 2x product op per <=4-col
        # run, ScalarE Copy-accum reduces stream behind it per column.
        for c0, cw in quads:
            prod = prp.tile([128, cw * D], _F16, name=f"prod{c0}")
            nc.vector.tensor_tensor(
                out=prod[:], in0=xcol(s, c0, cw),
                in1=u_sb[:, 0:cw * D], op=mybir.AluOpType.mult)
            for j in range(cw):
                i = s * ncols + c0 + j
                dump32 = scr.tile([128, D], _F32, name="dump32")
                nc.scalar.activation(
                    dump32[:], prod[:, j * D:(j + 1) * D],
                    mybir.ActivationFunctionType.Copy,
                    accum_out=s_sb[:, i:i + 1])
        # Remaining columns fused on DVE (1x scalar_tensor_tensor w/ accum).
        for c in range(na, ncols):
            i = s * ncols + c
            dump = scr.tile([128, D], _F16, name="dump")
            nc.vector.scalar_tensor_tensor(
                out=dump[:], in0=xcol(s, c),
                scalar=0.0, in1=u_sb[:, 0:D],
                op0=mybir.AluOpType.add, op1=mybir.AluOpType.mult,
                accum_out=s_sb[:, i:i + 1])
        # Masking needs no ops: the host writes padding rows as
        # x_pad = -kappa*u/||u||^2, so their score is exactly -kappa
        # (exp -> 0) and their pooling contribution is e*x = 0*x_pad = 0.
        # e = exp(s - C), one batched op per sample, fp16 out for the PE.
        nc.scalar.activation(e_sb[:, s * ncols:(s + 1) * ncols],
                             s_sb[:, s * ncols:(s + 1) * ncols],
                             mybir.ActivationFunctionType.Exp,
                             bias=shift_sb[:])
        # Pooling: accumulate e_c . x_c into PSUM [1, D]; overlaps the next
        # sample's (and iteration's) score work.
        for c in range(ncols):
            i = s * ncols + c
            nc.tensor.matmul(
                pool_ps[s][:],
                e_sb[:, i:i + 1],
                xcol(s, c),
                start=(c == 0),
                stop=(c == ncols - 1),
            )
    nc.scalar.dma_start(out=eout[:], in_=e_sb[:])
    if not fin_top:
        _finalize()


def _get_program(ncols):
    if ncols not in _CACHE:
        _CACHE[ncols] = _build_program(ncols)
    return _CACHE[ncols]


def _prep_inputs(x, flat_mask, W, v):
    """Compact to valid rows, swizzle partition-major; (in_maps, meta)."""
    x = np.ascontiguousarray(x, dtype=np.float32)
    flat_mask = np.asarray(flat_mask)
    W = np.asarray(W, dtype=np.float32)
    v = np.asarray(v, dtype=np.float32)
    # scores = x @ u + (b . v); the constant is dropped by softmax invariance.
    u = (v @ W).astype(np.float16)
    # replicated 4x along free dim for the 4-column fused product op
    u_rep = np.ascontiguousarray(
        np.broadcast_to(np.tile(u, 4), (128, 4 * D)), dtype=np.float16)

    idxs = [np.nonzero(flat_mask[b] == 1)[0] for b in range(B)]
    counts = np.array([len(ix) for ix in idxs])
    ncols = max(1, int(-(-counts.max() // 128)))
    ncap = ncols * 128

    # Masking without any device ops: padding rows are set to
    # x_pad = -(KAPPA/||u||^2) * u, so their score is exactly -KAPPA
    # (exp -> 0 in fp32) and their pooling term is e*x = 0*x_pad = 0.
    # |x_pad . u elementwise| <= KAPPA, safely inside fp16 range.
    u64 = u.astype(np.float64)
    unorm2 = float((u64 * u64).sum())
    KAPPA = 1.0e4
    degenerate = not (unorm2 > 0.0
                      and KAPPA * float(np.abs(u64).max()) / unorm2 < 6.0e4)
    if degenerate:
        x_pad = np.zeros((D,), dtype=np.float16)
    else:
        x_pad = (-(KAPPA / unorm2) * u64).astype(np.float16)

    xc = np.empty((B, ncap, D), dtype=np.float16)
    for b in range(B):
        cnt = counts[b]
        if cnt:
            xc[b, :cnt] = x[b, idxs[b]]
        xc[b, cnt:] = x_pad
    # row = col*128 + p  ->  [B, 128, ncols, D] partition-major
    xc = xc.reshape(B, ncols, 128, D).transpose(0, 2, 1, 3)

    in_maps = []
    for core in range(N_CORES):
        lo = core * SPB
        in_maps.append({
            # [128, SPB*ncols*D] flat partition-major
            "x": np.ascontiguousarray(
                xc[lo:lo + SPB].transpose(1, 0, 2, 3)).reshape(128, -1),
            "u": u_rep,
        })
    meta = {"ncols": ncols, "mask_in_stt": False, "counts": counts,
            "degenerate": degenerate}
    return in_maps, meta


def kernel(x, flat_mask, W, b, v, **_unused):
    in_maps, meta = _prep_inputs(x, flat_mask, W, v)
    nc = _get_program(meta["ncols"])
    res = run_bass_kernel_spmd(nc, in_maps, core_ids=list(range(N_CORES)))
    raw = np.concatenate([res.results[i]["out"] for i in range(N_CORES)],
                         axis=0)
    nct = in_maps[0]["x"].shape[1] // (SPB * D)
    z = np.concatenate(
        [res.results[i]["eout"].reshape(128, SPB, nct)
         .astype(np.float32).sum(axis=(0, 2))
         for i in range(N_CORES)], axis=0)
    out = (raw / z[:, None]).astype(np.float32)
    if (meta["counts"] == 0).any():
        # Reference semantics for an all-masked sample: uniform mean pool.
        x = np.asarray(x, dtype=np.float32)
        for bi in np.nonzero(meta["counts"] == 0)[0]:
            out[bi] = x[bi].mean(axis=0)
    if meta["degenerate"]:
        # Near-zero or pathological u = v@W: the x_pad masking trick can't
        # represent the padding rows in fp16. Tiny host fallback (never
        # triggers for randn-scale inputs).
        x = np.asarray(x, dtype=np.float32)
        u = (np.asarray(v, np.float64) @ np.asarray(W, np.float64))
        for bi in range(B):
            m = np.asarray(flat_mask[bi]) == 1
            if not m.any():
                continue
            s = x[bi, m].astype(np.float64) @ u
            w = np.exp(s - s.max())
            w /= w.sum()
            out[bi] = (w[:, None] * x[bi, m]).sum(0).astype(np.float32)
    return out
